# revision 21
# baseline (speedup 1.0000x reference)
"""MCAM kernel (per-core program), v5.

Per core (one sample b):
  f_b = W_b @ x_b   (1x1 conv, fp32r matmuls, f32 PSUM) -> f16_b fp16 [c | pix]
  G tiles (per mpar): [p=(mhalf,h) | w 64, c_l 128] built via PAIRED PE
      transposes (tile_position col groups 0/64) -> full-128-partition
      CONTIGUOUS fp16 PSUM->SBUF evacs.  Dummy matmuls are interleaved to
      keep the PE HAM clock-gate warm (transpose-mode does not count).
  Gram: per channel step 4 MMs on the two DIAGONAL quadrants (off-diagonal
      tile_position crashes HW), S[p=(half,i) | j 64, slot 256]; channel
      c = m*128 + (slot%128) with m = half*2 + (slot//128).  Running max
      over c comes from the gram PSUM tiles (DVE tensor_reduce) so softmax
      starts right after gram.
  Softmax: E = exp(S - M) on ACT (bias fused, accum_out -> Z); branch o's
      exps are interleaved into branch-s gram emission so ACT works during
      gram_s.  Branch s exp -> eb chunks, DVE TT multiplies into had.
  (had*rc)^2: DVE TS (had*rc -> eb, reusing the chunk tile) + DVE TT square
      back into had.
  PE-transpose had2 back to [c | pix] (row-half concurrent), combine
      att = (had2 * f16_o) * f16_s in fp16, DMA out fp16 (host casts f32).

f16_o/f16_s spill to DRAM after their G-transposes and reload before the
combine so S_o + S_s + G + had fit in SBUF.  Right SBUF stack: S_s is
allocated FIRST (below S_o) so S_o can close as soon as exp_o drains while
S_s lives on through the softmax chunks.
"""
from contextlib import ExitStack

import numpy as np

import concourse.bass as bass
import concourse.bacc as bacc
import concourse.mybir as mybir
import concourse.tile as tile
from concourse.masks import make_identity

F32 = mybir.dt.float32
F32R = mybir.dt.float32r
F16 = mybir.dt.float16
AL = mybir.AluOpType
AF = mybir.ActivationFunctionType
AX = mybir.AxisListType

C, HH, WW = 512, 64, 64
PIX = HH * WW  # 4096
NM = 4
NK = 4
NSLAB = 8
PITCH = 64 * 256  # S free-pitch per partition: [j 64, slot 256]


def rap(t, dims, off=0):
    return bass.AP(tensor=t.tensor, offset=t.offset + off, ap=[list(d) for d in dims])


def build_core():
    nc = bacc.Bacc("TRN2", target_bir_lowering=False, debug=False)
    x_dram = {
        "o": nc.dram_tensor("x_opt", [C, PIX], F32R, kind="ExternalInput").ap(),
        "s": nc.dram_tensor("x_sar", [C, PIX], F32R, kind="ExternalInput").ap(),
    }
    w_dram = {
        "o": nc.dram_tensor("w_opt", [C, C], F32, kind="ExternalInput").ap(),
        "s": nc.dram_tensor("w_sar", [C, C], F32, kind="ExternalInput").ap(),
    }
    att = nc.dram_tensor("att", [C, PIX], F16, kind="ExternalOutput").ap()
    spill = {
        "o": nc.dram_tensor("spill_o", [128, NM * PIX], F16, kind="Internal").ap(),
        "s": nc.dram_tensor("spill_s", [128, NM * PIX], F16, kind="Internal").ap(),
    }

    with tile.TileContext(nc) as tc, ExitStack() as ctx:
        persist = ctx.enter_context(tc.tile_pool(name="persist", bufs=1))
        smalls = ctx.enter_context(tc.tile_pool(name="smalls", bufs=1))
        cps = ctx.enter_context(tc.tile_pool(name="cps", bufs=2, space="PSUM"))
        dps = ctx.enter_context(tc.tile_pool(name="dps", bufs=1, space="PSUM"))

        ident = persist.tile([128, 128], F32, name="ident")
        make_identity(nc, ident)
        ident16 = persist.tile([128, 128], F16, name="ident16")
        make_identity(nc, ident16)

        # had: E_o, then E_o*E_s, finally (had*rc)^2, per chunk
        had = persist.tile([128, 64, 256], F16, name="had")
        Zp = {
            "o": smalls.tile([128, 64], F32, name="Zp_o"),
            "s": smalls.tile([128, 64], F32, name="Zp_s"),
        }
        Mrun = {
            "o": smalls.tile([128, 64], F32, name="Mrun_o"),
            "s": smalls.tile([128, 64], F32, name="Mrun_s"),
        }
        negM = {
            "o": smalls.tile([128, 64], F32, name="negM_o"),
            "s": smalls.tile([128, 64], F32, name="negM_s"),
        }
        Mpart = {
            "o": smalls.tile([128, 64], F32, name="Mpart_o"),
            "s": smalls.tile([128, 64], F32, name="Mpart_s"),
        }
        Zt_o = smalls.tile([64, 64], F32, name="Zt_o")

        def dummy_mm():
            """A real (non-transpose) matmul to keep the HAM clock warm."""
            dcp = dps.tile([128, 128], F32, tag="dcp")
            nc.tensor.matmul(
                dcp, lhsT=ident16, rhs=ident16, start=True, stop=True
            )

        def load_wt(b, pool, tc_=None):
            """WT[ci_p, k, co] = W[co, k*128+ci_p]"""
            WT = pool.tile([128, NK, C], F32R, tag="WT", name="WT")
            with tc.tile_pool(name=f"wsb_{b}", bufs=1) as wsbp:
                wsb = wsbp.tile([128, NM, C], F32, name="wsb")
                nc.sync.dma_start(
                    out=wsb, in_=w_dram[b].rearrange("(m p) ci -> p m ci", p=128)
                )
                for ko in range(NK):
                    wps = cps.tile([128, C], F32, tag="cp")
                    for mo in range(NM):
                        nc.tensor.transpose(
                            wps[:, mo * 128:(mo + 1) * 128],
                            in_=wsb[:, mo, ko * 128:(ko + 1) * 128],
                            identity=ident,
                        )
                    nc.scalar.copy(out=WT[:, ko, :], in_=wps)
            return WT

        def conv(b, f_out, WT, pool, evac_acts):
            """evac_acts: number of slabs (of NSLAB) evacuated on ACT."""
            for slab in range(NSLAB):
                xt = pool.tile([128, NK, 512], F32R, tag="xt")
                for k in range(NK):
                    nc.sync.dma_start(
                        out=xt[:, k, :],
                        in_=x_dram[b][k * 128:(k + 1) * 128,
                                      slab * 512:(slab + 1) * 512],
                    )
                for m in range(NM):
                    cp = cps.tile([128, 512], F32, tag="cp")
                    for k in range(NK):
                        nc.tensor.matmul(
                            cp,
                            lhsT=WT[:, k, m * 128:(m + 1) * 128],
                            rhs=xt[:, k, :],
                            start=(k == 0),
                            stop=(k == NK - 1),
                        )
                    sl = slice(slab * 512, (slab + 1) * 512)
                    if slab < evac_acts:
                        nc.scalar.copy(out=f_out[:, m, sl], in_=cp)
                    else:
                        nc.vector.tensor_copy(out=f_out[:, m, sl], in_=cp)

        def gtranspose(b, f16_t, G2, tps, dve_evacs):
            """G2[mpar][p=(mhalf,h) | w 64, c_l 128]; mhalf 0 -> m=mpar,
            mhalf 1 -> m=mpar+2.  Paired transposes (col groups 0/64) fill
            all 128 partitions; evacs contiguous fp16 [128, 1024]."""
            step = 0
            for mpar in range(2):
                for wq in range(8):
                    tp = tps.tile([128, 8, 128], F16, tag="tp")
                    for wi in range(8):
                        w = wq * 8 + wi
                        for half, m in ((0, mpar), (1, mpar + 2)):
                            src = rap(
                                f16_t[:, m, :], [[NM * PIX, 128], [WW, HH]],
                                off=w,
                            )
                            nc.tensor.transpose(
                                tp[half * 64:(half + 1) * 64, wi, :],
                                in_=src,
                                identity=ident16,
                                tile_position=(0, half * 64),
                            )
                    dummy_mm()  # HAM keepalive
                    dst = G2[mpar][:, wq * 8:(wq + 1) * 8, :]
                    if (step % 4) < dve_evacs:
                        nc.vector.tensor_copy(out=dst, in_=tp)
                    else:
                        nc.scalar.copy(out=dst, in_=tp)
                    step += 1

        def gram(b, G2, S, gps, act_evac_frac, act_filler=None):
            """Diagonal-quadrant MMs (2 concurrent pairs per channel step);
            running max over c from the PSUM tiles.  act_filler(grp) lets
            the caller interleave ACT work into the evac stream."""
            Mr, Mp = Mrun[b], Mpart[b]
            for grp in range(16):
                gp = gps.tile([128, 16, 64], F32, tag="gp")
                for t in range(8):
                    kl = grp * 8 + t
                    for gidx in range(2):
                        for mh in range(2):
                            lhs = G2[gidx][mh * 64:(mh + 1) * 64, :, kl]
                            # S-half = mh, slot-block = gidx
                            nc.tensor.matmul(
                                gp[mh * 64:(mh + 1) * 64, 2 * t + gidx, :],
                                lhsT=lhs,
                                rhs=lhs,
                                start=True,
                                stop=True,
                            )
                tr_in = rap(gp, [[1024, 128], [1, 64], [64, 16]])
                if grp == 0:
                    nc.vector.tensor_reduce(out=Mr, in_=tr_in, axis=AX.X, op=AL.max)
                else:
                    nc.vector.tensor_reduce(out=Mp, in_=tr_in, axis=AX.X, op=AL.max)
                    nc.vector.tensor_tensor(out=Mr, in0=Mr, in1=Mp, op=AL.max)
                for sb in range(2):
                    src = rap(gp, [[1024, 128], [128, 8], [1, 64]], off=sb * 64)
                    dst = rap(
                        S, [[PITCH, 128], [1, 8], [256, 64]],
                        off=sb * 128 + grp * 8,
                    )
                    if (grp % 4) < act_evac_frac * 4:
                        nc.scalar.copy(out=dst, in_=src)
                    else:
                        nc.vector.tensor_copy(out=dst, in_=src)
                if act_filler is not None:
                    act_filler(grp)

        def build_negM(b):
            tmp = smalls.tile([64, 64], F32, name=f"nmt_{b}")
            nc.scalar.copy(out=tmp, in_=Mrun[b][64:128])
            nc.vector.tensor_tensor(
                out=negM[b][0:64], in0=Mrun[b][0:64], in1=tmp, op=AL.max
            )
            nc.vector.tensor_scalar_mul(
                out=negM[b][0:64], in0=negM[b][0:64], scalar1=-1.0
            )
            nc.scalar.copy(out=negM[b][64:128], in_=negM[b][0:64])

        # ================= schedule =================
        # --- branch o: conv, G-transpose (+spill) ---
        Go_cm = tc.tile_pool(name="G_o", bufs=1, side="right")
        gpool_o = Go_cm.__enter__()
        G2o = [
            gpool_o.tile([128, 64, 128], F16, name=f"G_o{i}") for i in range(2)
        ]
        with tc.tile_pool(name="f16o", bufs=1) as f16o_pool:
            f16_o = f16o_pool.tile([128, NM, PIX], F16, name="f16_o")
            with (
                tc.tile_pool(name="w_o", bufs=1) as wp,
                tc.tile_pool(name="xt_o", bufs=2) as xw,
            ):
                WT = load_wt("o", wp)
                conv("o", f16_o, WT, xw, evac_acts=4)
            with tc.tile_pool(name="tps_o", bufs=2, space="PSUM") as tps_o:
                gtranspose("o", f16_o, G2o, tps_o, dve_evacs=2)
            for m in range(NM):
                nc.sync.dma_start(
                    out=spill["o"][:, m * PIX:(m + 1) * PIX],
                    in_=f16_o[:, m, :],
                )

        So_cm = tc.tile_pool(name="S_o", bufs=1)
        So_p = So_cm.__enter__()
        S_o = So_p.tile([128, 64, 256], F32, name="S_o")

        def exp_o_group(j0, n):
            for j in range(j0, j0 + n):
                nc.scalar.activation(
                    out=had[:, j, :],
                    in_=S_o[:, j, :],
                    func=AF.Exp,
                    bias=negM["o"][:, j:j + 1],
                    accum_out=Zp["o"][:, j:j + 1],
                )

        with tc.tile_pool(name="gps_o", bufs=2, space="PSUM") as gps_o:
            gram("o", G2o, S_o, gps_o, act_evac_frac=0.5)
        Go_cm.__exit__(None, None, None)

        # --- branch s conv/gtp; softmax_o interleaves into gram_s ---
        Gs_cm = tc.tile_pool(name="G_s", bufs=1)
        gpool_s = Gs_cm.__enter__()
        G2s = [
            gpool_s.tile([128, 64, 128], F16, name=f"G_s{i}") for i in range(2)
        ]
        with tc.tile_pool(name="f16s", bufs=1) as f16s_pool:
            f16_s = f16s_pool.tile([128, NM, PIX], F16, name="f16_s")
            with (
                tc.tile_pool(name="w_s", bufs=1) as wp,
                tc.tile_pool(name="xt_s", bufs=2) as xw,
            ):
                WT = load_wt("s", wp)
                conv("s", f16_s, WT, xw, evac_acts=0)  # DVE evacs
            build_negM("o")
            with tc.tile_pool(name="tps_s", bufs=2, space="PSUM") as tps_s:
                gtranspose("s", f16_s, G2s, tps_s, dve_evacs=4)
            for m in range(NM):
                nc.sync.dma_start(
                    out=spill["s"][:, m * PIX:(m + 1) * PIX],
                    in_=f16_s[:, m, :],
                )

        Ss_cm = tc.tile_pool(name="S_s", bufs=1, side="right")
        S_s = Ss_cm.__enter__().tile([128, 64, 256], F32, name="S_s")
        with tc.tile_pool(name="gps_s", bufs=2, space="PSUM") as gps_s:
            # 64 exp_o insts ride along with the 16 gram groups
            gram("s", G2s, S_s, gps_s, act_evac_frac=0.0,
                 act_filler=lambda grp: exp_o_group(grp * 4, 4))
        Gs_cm.__exit__(None, None, None)

        t2 = smalls.tile([64, 64], F32, name="zt2_o")
        nc.scalar.copy(out=t2, in_=Zp["o"][64:128])
        nc.vector.tensor_tensor(out=Zt_o, in0=Zp["o"][0:64], in1=t2, op=AL.add)
        # S_o is drained (exp_o done); release before the tail allocs
        So_cm.__exit__(None, None, None)

        # --- reload f16 for the combine (DMA overlaps softmax_s) ---
        f16r_cm = tc.tile_pool(name="f16r", bufs=1, side="right")
        f16r_p = f16r_cm.__enter__()
        f16r = {}
        for b in "os":
            f16r[b] = f16r_p.tile([128, NM, PIX], F16, name=f"f16r_{b}")
            for m in range(NM):
                nc.sync.dma_start(
                    out=f16r[b][:, m, :],
                    in_=spill[b][:, m * PIX:(m + 1) * PIX],
                )
        hc_cm = tc.tile_pool(name="hc", bufs=1, side="right")
        hc_p = hc_cm.__enter__()
        hc = [hc_p.tile([128, PIX], F16, name=f"hc{m}") for m in range(NM)]

        # --- softmax_s + hadamard + (had*rc)^2 + transpose-back, chunked ---
        build_negM("s")
        with (
            tc.tile_pool(name="eb", bufs=2) as ebp,
            tc.tile_pool(name="tops", bufs=2, space="PSUM") as tops,
        ):
            for chk in range(8):  # chunks of 8 j
                j0 = chk * 8
                cols = slice(j0, j0 + 8)
                eb = ebp.tile([128, 8, 256], F16, tag="eb")
                for jj in range(8):
                    j = j0 + jj
                    nc.scalar.activation(
                        out=eb[:, jj, :],
                        in_=S_s[:, j, :],
                        func=AF.Exp,
                        bias=negM["s"][:, j:j + 1],
                        accum_out=Zp["s"][:, j:j + 1],
                    )
                hs = had[:, j0:j0 + 8, :]
                nc.vector.tensor_tensor(out=hs, in0=hs, in1=eb, op=AL.mult)
                # rc = 1/(Zo*Zs) for this chunk, both partition halves
                t2s = smalls.tile([64, 8], F32, name=f"t2s_{chk}")
                nc.scalar.copy(out=t2s, in_=Zp["s"][64:128, cols])
                zts = smalls.tile([64, 8], F32, name=f"zts_{chk}")
                nc.vector.tensor_tensor(
                    out=zts, in0=Zp["s"][0:64, cols], in1=t2s, op=AL.add
                )
                nc.vector.tensor_tensor(
                    out=zts, in0=zts, in1=Zt_o[:, cols], op=AL.mult
                )
                rc = smalls.tile([128, 8], F32, name=f"rc_{chk}")
                nc.vector.reciprocal(out=rc[0:64], in_=zts)
                nc.scalar.copy(out=rc[64:128], in_=rc[0:64])
                # had*rc -> eb (chunk tile is free now), square back into had
                for jj in range(8):
                    j = j0 + jj
                    nc.vector.tensor_scalar(
                        out=eb[:, jj, :],
                        in0=had[:, j, :],
                        scalar1=rc[:, jj:jj + 1],
                        scalar2=None,
                        op0=AL.mult,
                    )
                nc.vector.tensor_tensor(out=hs, in0=eb, in1=eb, op=AL.mult)
                # transpose-back this chunk: m = half*2 + sb
                for sb in range(2):
                    tpo = []
                    for h in range(2):
                        t_ = tops.tile([128, 8, 64], F16, tag=f"tpo{h}")
                        tpo.append(t_)
                    for jj in range(8):
                        j = j0 + jj
                        for half in range(2):
                            nc.tensor.transpose(
                                tpo[half][:, jj, :],
                                in_=had[half * 64:(half + 1) * 64, j,
                                        sb * 128:(sb + 1) * 128],
                                identity=ident16[half * 64:(half + 1) * 64,
                                                 half * 64:(half + 1) * 64],
                            )
                    for half in range(2):
                        m = half * 2 + sb
                        dst = rap(hc[m], [[PIX, 128], [1, 8], [64, 64]], off=j0)
                        srcap = rap(tpo[half], [[512, 128], [64, 8], [1, 64]])
                        if half == 0:
                            nc.scalar.copy(out=dst, in_=srcap)
                        else:
                            nc.vector.tensor_copy(out=dst, in_=srcap)

        # --- combine + DMA out (fp16; host casts to f32) ---
        with tc.tile_pool(name="apool", bufs=2) as apool:
            for m in range(NM):
                for ih in range(2):
                    psl = slice(ih * 2048, (ih + 1) * 2048)
                    vv = apool.tile([128, 2048], F16, tag="vv")
                    nc.vector.tensor_tensor(
                        out=vv, in0=hc[m][:, psl], in1=f16r["o"][:, m, psl],
                        op=AL.mult,
                    )
                    nc.vector.tensor_tensor(
                        out=vv, in0=vv, in1=f16r["s"][:, m, psl], op=AL.mult
                    )
                    nc.sync.dma_start(
                        out=att[m * 128:(m + 1) * 128, psl], in_=vv
                    )

        hc_cm.__exit__(None, None, None)
        f16r_cm.__exit__(None, None, None)
        Ss_cm.__exit__(None, None, None)

    nc.compile()
    return nc


_NC_CACHE = {}


def _get_nc():
    if "nc" not in _NC_CACHE:
        _NC_CACHE["nc"] = build_core()
    return _NC_CACHE["nc"]


def kernel(opt, sar, W_opt, W_sar):
    """Full inputs (8,512,64,64)x2 + (512,512)x2 -> full output (8,512,64,64).

    Data-parallel over batch: one sample per NeuronCore.
    """
    from concourse.bass_utils import run_bass_kernel_spmd

    B = opt.shape[0]
    nc = _get_nc()
    in_maps = [
        {
            "x_opt": np.ascontiguousarray(opt[b].reshape(C, PIX), dtype=np.float32),
            "x_sar": np.ascontiguousarray(sar[b].reshape(C, PIX), dtype=np.float32),
            "w_opt": np.ascontiguousarray(W_opt, dtype=np.float32),
            "w_sar": np.ascontiguousarray(W_sar, dtype=np.float32),
        }
        for b in range(B)
    ]
    res = run_bass_kernel_spmd(nc, in_maps, core_ids=list(range(B)))
    out = np.stack([res.results[b]["att"].reshape(C, HH, WW) for b in range(B)])
    return out.astype(np.float32)


# revision 22
# speedup vs baseline: 1.2735x; 1.2735x over previous
"""MCAM kernel (per-core program), v6.

Per core (one sample b):
  f_b = W_b @ x_b: 1x1 conv in PURE FP16 (host casts x/W to fp16; f32 PSUM
      accumulation keeps precision), f16_b in [c | pix].
  G tiles (per mpar): [p=(mhalf,h) | w 64, c_l 128] via PE transposes into
      two 64-partition PSUM tiles (tp0/tp1), evac'd with a partition-shift
      copy for the upper half.
  Gram: per channel step 4 MMs on the two DIAGONAL quadrants,
      S[p=(half,i) | j 64, slot 256]; channel c = m*128 + (slot%128),
      m = half*2 + (slot//128).  Running max over c from gram PSUM tiles.
  INTERLEAVING (the key to HAM warmth + overlap): low-duty PE phases are
      emission-interleaved with real work:
        gtp_o  x conv_s   (conv matmuls keep the PE clock-gate at 8/8)
        gram_o x gtp_s
        gram_s x exp_o    (ACT runs softmax_o while PE does gram_s)
  Softmax: E = exp(S - M) on ACT (bias fused, NO accumulator reads);
      Z via DVE tensor_scalar accum_out.  Branch s exp -> eb chunks,
      TT multiplies into had; (had*rc)^2 via TS into eb + TT square.
  PE-transpose had2 back to [c | pix], combine att = (had2*f16_o)*f16_s
      in fp16, DMA out fp16 (host casts back to f32).

f16_o/f16_s spill to DRAM after their G-transposes, reloaded during the
softmax tail for the combine.
"""
from contextlib import ExitStack

import numpy as np

import concourse.bass as bass
import concourse.bacc as bacc
import concourse.mybir as mybir
import concourse.tile as tile
from concourse.masks import make_identity

F32 = mybir.dt.float32
F16 = mybir.dt.float16
AL = mybir.AluOpType
AF = mybir.ActivationFunctionType
AX = mybir.AxisListType

C, HH, WW = 512, 64, 64
PIX = HH * WW  # 4096
NM = 4
NK = 4
NSLAB = 8
PITCH = 64 * 256  # S free-pitch per partition: [j 64, slot 256]


def rap(t, dims, off=0):
    return bass.AP(tensor=t.tensor, offset=t.offset + off, ap=[list(d) for d in dims])


def interleave(*gens):
    alive = list(gens)
    while alive:
        for g in list(alive):
            try:
                next(g)
            except StopIteration:
                alive.remove(g)


def build_core():
    nc = bacc.Bacc("TRN2", target_bir_lowering=False, debug=False)
    x_dram = {
        "o": nc.dram_tensor("x_opt", [C, PIX], F16, kind="ExternalInput").ap(),
        "s": nc.dram_tensor("x_sar", [C, PIX], F16, kind="ExternalInput").ap(),
    }
    w_dram = {
        "o": nc.dram_tensor("w_opt", [C, C], F16, kind="ExternalInput").ap(),
        "s": nc.dram_tensor("w_sar", [C, C], F16, kind="ExternalInput").ap(),
    }
    att = nc.dram_tensor("att", [C, PIX], F16, kind="ExternalOutput").ap()
    spill = {
        "o": nc.dram_tensor("spill_o", [128, NM * PIX], F16, kind="Internal").ap(),
        "s": nc.dram_tensor("spill_s", [128, NM * PIX], F16, kind="Internal").ap(),
    }

    with tile.TileContext(nc) as tc, ExitStack() as ctx:
        persist = ctx.enter_context(tc.tile_pool(name="persist", bufs=1))
        smalls = ctx.enter_context(tc.tile_pool(name="smalls", bufs=1))
        cps = ctx.enter_context(tc.tile_pool(name="cps", bufs=2, space="PSUM"))

        ident16 = persist.tile([128, 128], F16, name="ident16")
        make_identity(nc, ident16)

        Zp = {
            "o": smalls.tile([128, 64], F32, name="Zp_o"),
            "s": smalls.tile([128, 64], F32, name="Zp_s"),
        }
        Mrun = {
            "o": smalls.tile([128, 64], F32, name="Mrun_o"),
            "s": smalls.tile([128, 64], F32, name="Mrun_s"),
        }
        negM = {
            "o": smalls.tile([128, 64], F32, name="negM_o"),
            "s": smalls.tile([128, 64], F32, name="negM_s"),
        }
        Mpart = {
            "o": smalls.tile([128, 64], F32, name="Mpart_o"),
            "s": smalls.tile([128, 64], F32, name="Mpart_s"),
        }
        Zt_o = smalls.tile([64, 64], F32, name="Zt_o")
        zjunk = smalls.tile([128, 256], F16, name="zjunk")

        def load_wt(b, pool):
            """WT[ci_p, k, co] = W[co, k*128+ci_p] (fp16)."""
            WT = pool.tile([128, NK, C], F16, name="WT")
            with (
                tc.tile_pool(name=f"wsb_{b}", bufs=1) as wsbp,
                tc.tile_pool(name=f"wps_{b}", bufs=2, space="PSUM") as wpsp,
            ):
                wsb = wsbp.tile([128, NM, C], F16, name="wsb")
                nc.sync.dma_start(
                    out=wsb, in_=w_dram[b].rearrange("(m p) ci -> p m ci", p=128)
                )
                for ko in range(NK):
                    wps = wpsp.tile([128, C], F16, tag="wps")
                    for mo in range(NM):
                        nc.tensor.transpose(
                            wps[:, mo * 128:(mo + 1) * 128],
                            in_=wsb[:, mo, ko * 128:(ko + 1) * 128],
                            identity=ident16,
                        )
                    nc.scalar.copy(out=WT[:, ko, :], in_=wps)
            return WT

        def conv_gen(b, f_out, WT, pool, evac_acts):
            """fp16 matmuls, f32 PSUM.  Yields per (slab, m) block."""
            for slab in range(NSLAB):
                xt = pool.tile([128, NK, 512], F16, tag="xt")
                for k in range(NK):
                    nc.sync.dma_start(
                        out=xt[:, k, :],
                        in_=x_dram[b][k * 128:(k + 1) * 128,
                                      slab * 512:(slab + 1) * 512],
                    )
                for m in range(NM):
                    cp = cps.tile([128, 512], F32, tag="cp")
                    for k in range(NK):
                        nc.tensor.matmul(
                            cp,
                            lhsT=WT[:, k, m * 128:(m + 1) * 128],
                            rhs=xt[:, k, :],
                            start=(k == 0),
                            stop=(k == NK - 1),
                        )
                    sl = slice(slab * 512, (slab + 1) * 512)
                    if slab < evac_acts:
                        nc.scalar.copy(out=f_out[:, m, sl], in_=cp)
                    else:
                        nc.vector.tensor_copy(out=f_out[:, m, sl], in_=cp)
                    yield

        def gtp_gen(b, f16_t, G2, tps, dve_evacs):
            """G2[mpar][p=(mhalf,h) | w 64, c_l 128].  Two 64-partition
            PSUM tiles per wq; upper half evac is partition-shifted.
            Yields per wq."""
            step = 0
            for mpar in range(2):
                for wq in range(8):
                    tph = []
                    for h in range(2):
                        t_ = tps.tile([64, 8, 128], F16, tag=f"tp{h}")
                        tph.append(t_)
                    for wi in range(8):
                        w = wq * 8 + wi
                        for half, m in ((0, mpar), (1, mpar + 2)):
                            src = rap(
                                f16_t[:, m, :], [[NM * PIX, 128], [WW, HH]],
                                off=w,
                            )
                            nc.tensor.transpose(
                                tph[half][:, wi, :],
                                in_=src,
                                identity=ident16,
                            )
                    for half in range(2):
                        dst = G2[mpar][half * 64:(half + 1) * 64,
                                       wq * 8:(wq + 1) * 8, :]
                        if (step % 4) < dve_evacs:
                            nc.vector.tensor_copy(out=dst, in_=tph[half])
                        else:
                            nc.scalar.copy(out=dst, in_=tph[half])
                    step += 1
                    yield

        def gram_gen(b, G2, S, gps, act_evacs):
            """Diagonal-quadrant MMs; running max from PSUM; yields/grp."""
            Mr, Mp = Mrun[b], Mpart[b]
            for grp in range(16):
                gp = gps.tile([128, 16, 64], F32, tag="gp")
                for t in range(8):
                    kl = grp * 8 + t
                    for gidx in range(2):
                        for mh in range(2):
                            lhs = G2[gidx][mh * 64:(mh + 1) * 64, :, kl]
                            nc.tensor.matmul(
                                gp[mh * 64:(mh + 1) * 64, 2 * t + gidx, :],
                                lhsT=lhs,
                                rhs=lhs,
                                start=True,
                                stop=True,
                            )
                tr_in = rap(gp, [[1024, 128], [1, 64], [64, 16]])
                if grp == 0:
                    nc.vector.tensor_reduce(out=Mr, in_=tr_in, axis=AX.X, op=AL.max)
                else:
                    nc.vector.tensor_reduce(out=Mp, in_=tr_in, axis=AX.X, op=AL.max)
                    nc.vector.tensor_tensor(out=Mr, in0=Mr, in1=Mp, op=AL.max)
                for sb in range(2):
                    src = rap(gp, [[1024, 128], [128, 8], [1, 64]], off=sb * 64)
                    dst = rap(
                        S, [[PITCH, 128], [1, 8], [256, 64]],
                        off=sb * 128 + grp * 8,
                    )
                    if (grp % 4) < act_evacs:
                        nc.scalar.copy(out=dst, in_=src)
                    else:
                        nc.vector.tensor_copy(out=dst, in_=src)
                yield

        def build_negM(b):
            tmp = smalls.tile([64, 64], F32, name=f"nmt_{b}")
            nc.scalar.copy(out=tmp, in_=Mrun[b][64:128])
            nc.vector.tensor_tensor(
                out=negM[b][0:64], in0=Mrun[b][0:64], in1=tmp, op=AL.max
            )
            nc.vector.tensor_scalar_mul(
                out=negM[b][0:64], in0=negM[b][0:64], scalar1=-1.0
            )
            nc.scalar.copy(out=negM[b][64:128], in_=negM[b][0:64])

        # ================= schedule =================
        # R-stack: G_o, f16s -> had, S_s, f16r, hc
        Go_cm = tc.tile_pool(name="G_o", bufs=1, side="right")
        gpool_o = Go_cm.__enter__()
        G2o = [
            gpool_o.tile([128, 64, 128], F16, name=f"G_o{i}") for i in range(2)
        ]

        with tc.tile_pool(name="f16o", bufs=1) as f16o_pool:
            f16_o = f16o_pool.tile([128, NM, PIX], F16, name="f16_o")
            with (
                tc.tile_pool(name="w_o", bufs=1) as wpo,
                tc.tile_pool(name="xt_o", bufs=2) as xwo,
            ):
                WT_o = load_wt("o", wpo)
                for _ in conv_gen("o", f16_o, WT_o, xwo, evac_acts=4):
                    pass

            # f16_s lives on the right stack (outlives f16o's scope)
            f16s_cm = tc.tile_pool(name="f16s", bufs=1, side="right")
            f16_s = f16s_cm.__enter__().tile([128, NM, PIX], F16, name="f16_s")

            with (
                tc.tile_pool(name="w_s", bufs=1) as wps_,
                tc.tile_pool(name="xt_s", bufs=2) as xws,
                tc.tile_pool(name="tps_o", bufs=1, space="PSUM") as tps_o,
            ):
                WT_s = load_wt("s", wps_)
                interleave(
                    gtp_gen("o", f16_o, G2o, tps_o, dve_evacs=2),
                    conv_gen("s", f16_s, WT_s, xws, evac_acts=2),
                )
            for m in range(NM):
                nc.sync.dma_start(
                    out=spill["o"][:, m * PIX:(m + 1) * PIX],
                    in_=f16_o[:, m, :],
                )

        So_cm = tc.tile_pool(name="S_o", bufs=1)
        S_o = So_cm.__enter__().tile([128, 64, 256], F32, name="S_o")

        Gs_cm = tc.tile_pool(name="G_s", bufs=1)
        gpool_s = Gs_cm.__enter__()
        G2s = [
            gpool_s.tile([128, 64, 128], F16, name=f"G_s{i}") for i in range(2)
        ]

        with (
            tc.tile_pool(name="gps_o", bufs=2, space="PSUM") as gps_o,
            tc.tile_pool(name="tps_s", bufs=1, space="PSUM") as tps_s,
        ):
            interleave(
                gram_gen("o", G2o, S_o, gps_o, act_evacs=4),
                gtp_gen("s", f16_s, G2s, tps_s, dve_evacs=4),
            )
        for m in range(NM):
            nc.sync.dma_start(
                out=spill["s"][:, m * PIX:(m + 1) * PIX],
                in_=f16_s[:, m, :],
            )
        f16s_cm.__exit__(None, None, None)
        Go_cm.__exit__(None, None, None)

        # had + S_s on the right stack for the tail
        had_cm = tc.tile_pool(name="hadp", bufs=1, side="right")
        had = had_cm.__enter__().tile([128, 64, 256], F16, name="had")
        Ss_cm = tc.tile_pool(name="S_s", bufs=1, side="right")
        S_s = Ss_cm.__enter__().tile([128, 64, 256], F32, name="S_s")

        build_negM("o")

        def exp_o_gen():
            for g in range(16):
                for j in range(g * 4, g * 4 + 4):
                    nc.scalar.activation(
                        out=had[:, j, :],
                        in_=S_o[:, j, :],
                        func=AF.Exp,
                        bias=negM["o"][:, j:j + 1],
                    )
                yield

        with tc.tile_pool(name="gps_s", bufs=2, space="PSUM") as gps_s:
            interleave(
                gram_gen("s", G2s, S_s, gps_s, act_evacs=2),
                exp_o_gen(),
            )

        # Z_o = sum_c E_o via DVE TS accum (frees ACT of accumulator reads)
        for j in range(64):
            nc.vector.tensor_scalar(
                out=zjunk,
                in0=had[:, j, :],
                scalar1=1.0,
                scalar2=None,
                op0=AL.mult,
                op1=AL.add,
                accum_out=Zp["o"][:, j:j + 1],
            )
        t2 = smalls.tile([64, 64], F32, name="zt2_o")
        nc.scalar.copy(out=t2, in_=Zp["o"][64:128])
        nc.vector.tensor_tensor(out=Zt_o, in0=Zp["o"][0:64], in1=t2, op=AL.add)
        Gs_cm.__exit__(None, None, None)
        So_cm.__exit__(None, None, None)

        # --- reload f16 for the combine (DMA overlaps softmax_s) ---
        f16r_cm = tc.tile_pool(name="f16r", bufs=1, side="right")
        f16r_p = f16r_cm.__enter__()
        f16r = {}
        for b in "os":
            f16r[b] = f16r_p.tile([128, NM, PIX], F16, name=f"f16r_{b}")
            for m in range(NM):
                nc.sync.dma_start(
                    out=f16r[b][:, m, :],
                    in_=spill[b][:, m * PIX:(m + 1) * PIX],
                )
        hc_cm = tc.tile_pool(name="hc", bufs=1, side="right")
        hc_p = hc_cm.__enter__()
        hc = [hc_p.tile([128, PIX], F16, name=f"hc{m}") for m in range(NM)]

        # --- softmax_s + hadamard + (had*rc)^2 + transpose-back, chunked ---
        build_negM("s")
        with (
            tc.tile_pool(name="eb", bufs=2) as ebp,
            tc.tile_pool(name="tops", bufs=2, space="PSUM") as tops,
        ):
            for chk in range(8):  # chunks of 8 j
                j0 = chk * 8
                cols = slice(j0, j0 + 8)
                eb = ebp.tile([128, 8, 256], F16, tag="eb")
                for jj in range(8):
                    j = j0 + jj
                    nc.scalar.activation(
                        out=eb[:, jj, :],
                        in_=S_s[:, j, :],
                        func=AF.Exp,
                        bias=negM["s"][:, j:j + 1],
                    )
                # Z_s for this chunk on DVE
                for jj in range(8):
                    j = j0 + jj
                    nc.vector.tensor_scalar(
                        out=zjunk,
                        in0=eb[:, jj, :],
                        scalar1=1.0,
                        scalar2=None,
                        op0=AL.mult,
                        op1=AL.add,
                        accum_out=Zp["s"][:, j:j + 1],
                    )
                hs = had[:, j0:j0 + 8, :]
                nc.vector.tensor_tensor(out=hs, in0=hs, in1=eb, op=AL.mult)
                # rc = 1/(Zo*Zs) for this chunk, both partition halves
                t2s = smalls.tile([64, 8], F32, name=f"t2s_{chk}")
                nc.scalar.copy(out=t2s, in_=Zp["s"][64:128, cols])
                zts = smalls.tile([64, 8], F32, name=f"zts_{chk}")
                nc.vector.tensor_tensor(
                    out=zts, in0=Zp["s"][0:64, cols], in1=t2s, op=AL.add
                )
                nc.vector.tensor_tensor(
                    out=zts, in0=zts, in1=Zt_o[:, cols], op=AL.mult
                )
                rc = smalls.tile([128, 8], F32, name=f"rc_{chk}")
                nc.vector.reciprocal(out=rc[0:64], in_=zts)
                nc.scalar.copy(out=rc[64:128], in_=rc[0:64])
                # had*rc -> eb (chunk tile is free now), square back into had
                for jj in range(8):
                    j = j0 + jj
                    nc.vector.tensor_scalar(
                        out=eb[:, jj, :],
                        in0=had[:, j, :],
                        scalar1=rc[:, jj:jj + 1],
                        scalar2=None,
                        op0=AL.mult,
                    )
                nc.vector.tensor_tensor(out=hs, in0=eb, in1=eb, op=AL.mult)
                # transpose-back this chunk: m = half*2 + sb
                for sb in range(2):
                    tpo = []
                    for h in range(2):
                        t_ = tops.tile([128, 8, 64], F16, tag=f"tpo{h}")
                        tpo.append(t_)
                    for jj in range(8):
                        j = j0 + jj
                        for half in range(2):
                            nc.tensor.transpose(
                                tpo[half][:, jj, :],
                                in_=had[half * 64:(half + 1) * 64, j,
                                        sb * 128:(sb + 1) * 128],
                                identity=ident16[half * 64:(half + 1) * 64,
                                                 half * 64:(half + 1) * 64],
                            )
                    for half in range(2):
                        m = half * 2 + sb
                        dst = rap(hc[m], [[PIX, 128], [1, 8], [64, 64]], off=j0)
                        srcap = rap(tpo[half], [[512, 128], [64, 8], [1, 64]])
                        if half == 0:
                            nc.scalar.copy(out=dst, in_=srcap)
                        else:
                            nc.vector.tensor_copy(out=dst, in_=srcap)

        # --- combine + DMA out (fp16; host casts to f32) ---
        with tc.tile_pool(name="apool", bufs=2) as apool:
            for m in range(NM):
                for ih in range(2):
                    psl = slice(ih * 2048, (ih + 1) * 2048)
                    vv = apool.tile([128, 2048], F16, tag="vv")
                    nc.vector.tensor_tensor(
                        out=vv, in0=hc[m][:, psl], in1=f16r["o"][:, m, psl],
                        op=AL.mult,
                    )
                    nc.vector.tensor_tensor(
                        out=vv, in0=vv, in1=f16r["s"][:, m, psl], op=AL.mult
                    )
                    nc.sync.dma_start(
                        out=att[m * 128:(m + 1) * 128, psl], in_=vv
                    )

        hc_cm.__exit__(None, None, None)
        f16r_cm.__exit__(None, None, None)
        Ss_cm.__exit__(None, None, None)
        had_cm.__exit__(None, None, None)

    nc.compile()
    return nc


_NC_CACHE = {}


def _get_nc():
    if "nc" not in _NC_CACHE:
        _NC_CACHE["nc"] = build_core()
    return _NC_CACHE["nc"]


def kernel(opt, sar, W_opt, W_sar):
    """Full inputs (8,512,64,64)x2 + (512,512)x2 -> full output (8,512,64,64).

    Data-parallel over batch: one sample per NeuronCore.  x and W are cast
    to fp16 host-side (the conv runs fp16 with f32 PSUM accumulation); the
    fp16 output is cast back to f32 host-side.
    """
    from concourse.bass_utils import run_bass_kernel_spmd

    B = opt.shape[0]
    nc = _get_nc()
    in_maps = [
        {
            "x_opt": np.ascontiguousarray(opt[b].reshape(C, PIX)).astype(np.float16),
            "x_sar": np.ascontiguousarray(sar[b].reshape(C, PIX)).astype(np.float16),
            "w_opt": np.asarray(W_opt, dtype=np.float16),
            "w_sar": np.asarray(W_sar, dtype=np.float16),
        }
        for b in range(B)
    ]
    res = run_bass_kernel_spmd(nc, in_maps, core_ids=list(range(B)))
    out = np.stack([res.results[b]["att"].reshape(C, HH, WW) for b in range(B)])
    return out.astype(np.float32)


# revision 27
# speedup vs baseline: 1.3862x; 1.0885x over previous
"""MCAM kernel (per-core program), v7.

Layout trick: pix is stored W-MAJOR on device (x fed as [c, w*64+h] by the
host; att returned as [c, w*64+h] and un-permuted host-side).  This makes
both big data transposes expressible as XBAR DMA-transposes (2-byte dtype,
contiguous source), so they run on the (otherwise idle) DMA engines:
  G build:  f16 [c | w,h] -> G [h | w, c]  (4 transposes per branch)
  tp-back:  had2 [(half,i) | j, slot] -> hc [c | j, i]  (4 per chunk)
The PE does only convs + grams (dense, stays warm); ACT does exps; DVE the
hadamard/square and evacs; GPSIMD computes the softmax Z sums.

Pipeline: conv_o, G-dma_o/spill_o, conv_s, G-dma_s/spill_s, gram_o,
gram_s (+exp_o interleaved on ACT), then j-chunked tail:
exp_s -> Z(gpsimd) -> had *= eb -> rc -> (had*rc)^2 -> DMA-tpback,
then combine att = (had2 * f16_o) * f16_s (all fp16, w-major) and DMA out.
"""
from contextlib import ExitStack

import numpy as np

import concourse.bass as bass
import concourse.bacc as bacc
import concourse.mybir as mybir
import concourse.tile as tile
from concourse.masks import make_identity

F32 = mybir.dt.float32
F16 = mybir.dt.float16
AL = mybir.AluOpType
AF = mybir.ActivationFunctionType
AX = mybir.AxisListType

C, HH, WW = 512, 64, 64
PIX = HH * WW  # 4096
NM = 4
NK = 4
NSLAB = 8
PITCH = 64 * 256  # S free-pitch per partition: [j 64, slot 256]


def rap(t, dims, off=0):
    return bass.AP(tensor=t.tensor, offset=t.offset + off, ap=[list(d) for d in dims])


def interleave(*gens):
    alive = list(gens)
    while alive:
        for g in list(alive):
            try:
                next(g)
            except StopIteration:
                alive.remove(g)


def build_core():
    nc = bacc.Bacc("TRN2", target_bir_lowering=False, debug=False)
    # x is [c, w*64 + h] (w-major pix), fp16, prepared host-side
    x_dram = {
        "o": nc.dram_tensor("x_opt", [C, PIX], F16, kind="ExternalInput").ap(),
        "s": nc.dram_tensor("x_sar", [C, PIX], F16, kind="ExternalInput").ap(),
    }
    w_dram = {
        "o": nc.dram_tensor("w_opt", [C, C], F16, kind="ExternalInput").ap(),
        "s": nc.dram_tensor("w_sar", [C, C], F16, kind="ExternalInput").ap(),
    }
    # att is [c, w*64 + h] fp16; host casts + un-permutes
    att = nc.dram_tensor("att", [C, PIX], F16, kind="ExternalOutput").ap()
    # spill doubles as G staging: addr = gidx*8192 + w*128 + mh*64 + h
    spill = {
        "o": nc.dram_tensor("spill_o", [128, NM * PIX], F16, kind="Internal").ap(),
        "s": nc.dram_tensor("spill_s", [128, NM * PIX], F16, kind="Internal").ap(),
    }

    def spill_ap(b, m):
        gidx, mh = m % 2, m // 2
        return rap(
            spill[b], [[NM * PIX, 128], [128, 64], [1, 64]],
            off=gidx * 8192 + mh * 64,
        )

    with tile.TileContext(nc) as tc, ExitStack() as ctx:
        persist = ctx.enter_context(tc.tile_pool(name="persist", bufs=1))
        smalls = ctx.enter_context(tc.tile_pool(name="smalls", bufs=1))
        cps = ctx.enter_context(tc.tile_pool(name="cps", bufs=2, space="PSUM"))

        ident16 = persist.tile([128, 128], F16, name="ident16")
        make_identity(nc, ident16)

        Zp = {
            "o": smalls.tile([128, 64], F32, name="Zp_o"),
            "s": smalls.tile([128, 64], F32, name="Zp_s"),
        }
        Mrun = {
            "o": smalls.tile([128, 64], F32, name="Mrun_o"),
            "s": smalls.tile([128, 64], F32, name="Mrun_s"),
        }
        negM = {
            "o": smalls.tile([128, 64], F32, name="negM_o"),
            "s": smalls.tile([128, 64], F32, name="negM_s"),
        }
        Mpart = {
            "o": smalls.tile([128, 64], F32, name="Mpart_o"),
            "s": smalls.tile([128, 64], F32, name="Mpart_s"),
        }
        Zt_o = smalls.tile([64, 64], F32, name="Zt_o")
        zjunk = smalls.tile([128, 256], F16, name="zjunk")

        def load_wt(b, pool):
            """WT[ci_p, k, co] = W[co, k*128+ci_p] (fp16)."""
            WT = pool.tile([128, NK, C], F16, name="WT")
            with (
                tc.tile_pool(name=f"wsb_{b}", bufs=1) as wsbp,
                tc.tile_pool(name=f"wps_{b}", bufs=2, space="PSUM") as wpsp,
            ):
                wsb = wsbp.tile([128, NM, C], F16, name="wsb")
                nc.sync.dma_start(
                    out=wsb, in_=w_dram[b].rearrange("(m p) ci -> p m ci", p=128)
                )
                for ko in range(NK):
                    wps = wpsp.tile([128, C], F16, tag="wps")
                    for mo in range(NM):
                        nc.tensor.transpose(
                            wps[:, mo * 128:(mo + 1) * 128],
                            in_=wsb[:, mo, ko * 128:(ko + 1) * 128],
                            identity=ident16,
                        )
                    nc.scalar.copy(out=WT[:, ko, :], in_=wps)
            return WT

        def conv(b, f_out, WT, pool, evac_acts):
            """fp16 matmuls, f32 PSUM accumulation."""
            for slab in range(NSLAB):
                xt = pool.tile([128, NK, 512], F16, tag="xt")
                for k in range(NK):
                    nc.sync.dma_start(
                        out=xt[:, k, :],
                        in_=x_dram[b][k * 128:(k + 1) * 128,
                                      slab * 512:(slab + 1) * 512],
                    )
                for m in range(NM):
                    cp = cps.tile([128, 512], F32, tag="cp")
                    for k in range(NK):
                        nc.tensor.matmul(
                            cp,
                            lhsT=WT[:, k, m * 128:(m + 1) * 128],
                            rhs=xt[:, k, :],
                            start=(k == 0),
                            stop=(k == NK - 1),
                        )
                    sl = slice(slab * 512, (slab + 1) * 512)
                    if slab < evac_acts:
                        nc.scalar.copy(out=f_out[:, m, sl], in_=cp)
                    else:
                        nc.vector.tensor_copy(out=f_out[:, m, sl], in_=cp)

        def g_dma(b, G2):
            """G2[gidx][p=(mh,h) | w 64, c_l 128] = f[c, h, w] via XBAR DMA
            transpose of the DRAM staging (contiguous source, 128-part
            output -- the HW-verified configuration)."""
            for gidx in range(2):
                tsrc = rap(
                    spill[b], [[NM * PIX, 128], [1, 8192]], off=gidx * 8192
                )
                nc.sync.dma_start_transpose(out=G2[gidx][:, :, :], in_=tsrc)

        def gram_gen(b, G2, S, gps, act_evacs):
            """Diagonal-quadrant MMs; running max from PSUM; yields/grp."""
            Mr, Mp = Mrun[b], Mpart[b]
            for grp in range(16):
                gp = gps.tile([128, 16, 64], F32, tag="gp")
                for t in range(8):
                    kl = grp * 8 + t
                    for gidx in range(2):
                        for mh in range(2):
                            lhs = G2[gidx][mh * 64:(mh + 1) * 64, :, kl]
                            nc.tensor.matmul(
                                gp[mh * 64:(mh + 1) * 64, 2 * t + gidx, :],
                                lhsT=lhs,
                                rhs=lhs,
                                start=True,
                                stop=True,
                            )
                tr_in = rap(gp, [[1024, 128], [1, 64], [64, 16]])
                if grp == 0:
                    nc.vector.tensor_reduce(out=Mr, in_=tr_in, axis=AX.X, op=AL.max)
                else:
                    nc.vector.tensor_reduce(out=Mp, in_=tr_in, axis=AX.X, op=AL.max)
                    nc.vector.tensor_tensor(out=Mr, in0=Mr, in1=Mp, op=AL.max)
                for sb in range(2):
                    src = rap(gp, [[1024, 128], [128, 8], [1, 64]], off=sb * 64)
                    dst = rap(
                        S, [[PITCH, 128], [1, 8], [256, 64]],
                        off=sb * 128 + grp * 8,
                    )
                    if (grp % 4) < act_evacs:
                        nc.scalar.copy(out=dst, in_=src)
                    else:
                        nc.vector.tensor_copy(out=dst, in_=src)
                yield

        def build_negM(b):
            tmp = smalls.tile([64, 64], F32, name=f"nmt_{b}")
            nc.scalar.copy(out=tmp, in_=Mrun[b][64:128])
            nc.vector.tensor_tensor(
                out=negM[b][0:64], in0=Mrun[b][0:64], in1=tmp, op=AL.max
            )
            nc.vector.tensor_scalar_mul(
                out=negM[b][0:64], in0=negM[b][0:64], scalar1=-1.0
            )
            nc.scalar.copy(out=negM[b][64:128], in_=negM[b][0:64])

        # ================= schedule =================
        # L: [f16o{w_o,xt_o}] [G_s [S_o]] [eb] [apool]
        # R: [G_o, f16s] then [had, S_s, f16r, hc]
        Go_cm = tc.tile_pool(name="G_o", bufs=1, side="right")
        gpool_o = Go_cm.__enter__()
        G2o = [
            gpool_o.tile([128, 64, 128], F16, name=f"G_o{i}") for i in range(2)
        ]

        with tc.tile_pool(name="f16o", bufs=1) as f16o_pool:
            f16_o = f16o_pool.tile([128, NM, PIX], F16, name="f16_o")
            with (
                tc.tile_pool(name="w_o", bufs=1) as wpo,
                tc.tile_pool(name="xt_o", bufs=2) as xwo,
            ):
                WT_o = load_wt("o", wpo)
                conv("o", f16_o, WT_o, xwo, evac_acts=4)
            for m in range(NM):
                nc.sync.dma_start(out=spill_ap("o", m), in_=f16_o[:, m, :])
            g_dma("o", G2o)

        f16s_cm = tc.tile_pool(name="f16s", bufs=1, side="right")
        f16_s = f16s_cm.__enter__().tile([128, NM, PIX], F16, name="f16_s")
        with (
            tc.tile_pool(name="w_s", bufs=1) as wps_,
            tc.tile_pool(name="xt_s", bufs=2) as xws,
        ):
            WT_s = load_wt("s", wps_)
            conv("s", f16_s, WT_s, xws, evac_acts=2)

        Gs_cm = tc.tile_pool(name="G_s", bufs=1)
        gpool_s = Gs_cm.__enter__()
        G2s = [
            gpool_s.tile([128, 64, 128], F16, name=f"G_s{i}") for i in range(2)
        ]
        for m in range(NM):
            nc.sync.dma_start(out=spill_ap("s", m), in_=f16_s[:, m, :])
        g_dma("s", G2s)
        f16s_cm.__exit__(None, None, None)

        So_cm = tc.tile_pool(name="S_o", bufs=1)
        S_o = So_cm.__enter__().tile([128, 64, 256], F32, name="S_o")
        with tc.tile_pool(name="gps_o", bufs=3, space="PSUM") as gps_o:
            for _ in gram_gen("o", G2o, S_o, gps_o, act_evacs=2):
                pass
        Go_cm.__exit__(None, None, None)

        had_cm = tc.tile_pool(name="hadp", bufs=1, side="right")
        # had: [p | sb 2, j 64, sl 128] (slot-block major free layout)
        had = had_cm.__enter__().tile([128, 2, 64, 128], F16, name="had")

        def had_j(j, parts=None):
            t = had if parts is None else had[parts[0]:parts[1], :, :, :]
            n = 128 if parts is None else parts[1] - parts[0]
            return rap(t, [[16384, n], [8192, 2], [1, 128]], off=j * 128)

        def had_chunk(j0):
            return rap(had, [[16384, 128], [8192, 2], [1, 1024]], off=j0 * 128)
        Ss_cm = tc.tile_pool(name="S_s", bufs=1, side="right")
        S_s = Ss_cm.__enter__().tile([128, 64, 256], F32, name="S_s")

        build_negM("o")

        def exp_o_gen():
            for g in range(16):
                for j in range(g * 4, g * 4 + 4):
                    nc.scalar.activation(
                        out=had_j(j),
                        in_=S_o[:, j, :],
                        func=AF.Exp,
                        bias=negM["o"][:, j:j + 1],
                        accum_out=Zp["o"][:, j:j + 1],
                    )
                yield

        with tc.tile_pool(name="gps_s", bufs=3, space="PSUM") as gps_s:
            interleave(
                gram_gen("s", G2s, S_s, gps_s, act_evacs=0),
                exp_o_gen(),
            )
        t2 = smalls.tile([64, 64], F32, name="zt2_o")
        nc.scalar.copy(out=t2, in_=Zp["o"][64:128])
        nc.vector.tensor_tensor(out=Zt_o, in0=Zp["o"][0:64], in1=t2, op=AL.add)
        So_cm.__exit__(None, None, None)
        Gs_cm.__exit__(None, None, None)

        # --- reload f16 for the combine (DMA overlaps softmax_s) ---
        f16r_cm = tc.tile_pool(name="f16r", bufs=1, side="right")
        f16r_p = f16r_cm.__enter__()
        f16r = {}
        for b in "os":
            f16r[b] = f16r_p.tile([128, NM, PIX], F16, name=f"f16r_{b}")
            for m in range(NM):
                nc.sync.dma_start(out=f16r[b][:, m, :], in_=spill_ap(b, m))
        hc_cm = tc.tile_pool(name="hc", bufs=1, side="right")
        hc_p = hc_cm.__enter__()
        # hc[m]: [c_l | j 64, i 64]  (j-major, matching w-major f16)
        hc = [hc_p.tile([128, 64, 64], F16, name=f"hc{m}") for m in range(NM)]

        # --- softmax_s + hadamard + (had*rc)^2 + DMA transpose-back ---
        build_negM("s")
        with tc.tile_pool(name="eb", bufs=2) as ebp:
            for chk in range(8):  # chunks of 8 j
                j0 = chk * 8
                cols = slice(j0, j0 + 8)
                eb = ebp.tile([128, 2, 8, 128], F16, tag="eb")

                def eb_j(jj):
                    return rap(eb, [[2048, 128], [1024, 2], [1, 128]],
                               off=jj * 128)
                for jj in range(8):
                    j = j0 + jj
                    nc.scalar.activation(
                        out=eb_j(jj),
                        in_=S_s[:, j, :],
                        func=AF.Exp,
                        bias=negM["s"][:, j:j + 1],
                        accum_out=Zp["s"][:, j:j + 1],
                    )
                hs = had_chunk(j0)
                ebv = rap(eb, [[2048, 128], [1, 2048]])
                nc.vector.tensor_tensor(out=hs, in0=hs, in1=ebv, op=AL.mult)
                # rc = 1/(Zo*Zs) for this chunk, both partition halves
                t2s = smalls.tile([64, 8], F32, name=f"t2s_{chk}")
                nc.vector.tensor_copy(out=t2s, in_=Zp["s"][64:128, cols])
                zts = smalls.tile([64, 8], F32, name=f"zts_{chk}")
                nc.vector.tensor_tensor(
                    out=zts, in0=Zp["s"][0:64, cols], in1=t2s, op=AL.add
                )
                nc.vector.tensor_tensor(
                    out=zts, in0=zts, in1=Zt_o[:, cols], op=AL.mult
                )
                rc = smalls.tile([128, 8], F32, name=f"rc_{chk}")
                nc.vector.reciprocal(out=rc[0:64], in_=zts)
                nc.vector.tensor_copy(out=rc[64:128], in_=rc[0:64])
                # had*rc -> eb, square back into had
                for jj in range(8):
                    j = j0 + jj
                    nc.vector.tensor_scalar(
                        out=eb_j(jj),
                        in0=had_j(j),
                        scalar1=rc[:, jj:jj + 1],
                        scalar2=None,
                        op0=AL.mult,
                    )
                nc.vector.tensor_tensor(out=hs, in0=ebv, in1=ebv, op=AL.mult)
                # DMA transpose-back: had2[(half,i) | j-chunk, slot-block]
                # -> hc[m][:, j-chunk, :]   (m = half*2 + sb)
                for half in range(2):
                    for sb in range(2):
                        m = half * 2 + sb
                        tsrc = rap(
                            had[half * 64:(half + 1) * 64, :, :, :],
                            [[16384, 64], [128, 8], [1, 128]],
                            off=sb * 8192 + j0 * 128,
                        )
                        dst = hc[m][:, j0:j0 + 8, :]
                        nc.sync.dma_start_transpose(out=dst, in_=tsrc)

        # --- combine + DMA out (fp16 w-major; host casts + permutes) ---
        with tc.tile_pool(name="apool", bufs=2) as apool:
            for m in range(NM):
                for ih in range(2):
                    psl = slice(ih * 2048, (ih + 1) * 2048)
                    hcv = rap(hc[m], [[4096, 128], [1, 2048]], off=ih * 2048)
                    vv = apool.tile([128, 2048], F16, tag="vv")
                    nc.vector.tensor_tensor(
                        out=vv, in0=hcv, in1=f16r["o"][:, m, psl], op=AL.mult
                    )
                    nc.vector.tensor_tensor(
                        out=vv, in0=vv, in1=f16r["s"][:, m, psl], op=AL.mult
                    )
                    nc.sync.dma_start(
                        out=att[m * 128:(m + 1) * 128, psl], in_=vv
                    )

        hc_cm.__exit__(None, None, None)
        f16r_cm.__exit__(None, None, None)
        Ss_cm.__exit__(None, None, None)
        had_cm.__exit__(None, None, None)

    nc.compile()
    return nc


_NC_CACHE = {}


def _get_nc():
    if "nc" not in _NC_CACHE:
        _NC_CACHE["nc"] = build_core()
    return _NC_CACHE["nc"]


def _prep_x(x):
    """[C, H, W] f32 -> [C, PIX] fp16, w-major pix."""
    return np.ascontiguousarray(x.transpose(0, 2, 1).reshape(C, PIX)).astype(
        np.float16
    )


def kernel(opt, sar, W_opt, W_sar):
    """Full inputs (8,512,64,64)x2 + (512,512)x2 -> full output (8,512,64,64).

    Data-parallel over batch: one sample per NeuronCore.  x/W are cast to
    fp16 host-side and x is fed w-major; the fp16 w-major output is cast
    and permuted back host-side.
    """
    from concourse.bass_utils import run_bass_kernel_spmd

    B = opt.shape[0]
    nc = _get_nc()
    opt = np.asarray(opt)
    sar = np.asarray(sar)
    in_maps = [
        {
            "x_opt": _prep_x(opt[b]),
            "x_sar": _prep_x(sar[b]),
            "w_opt": np.asarray(W_opt, dtype=np.float16),
            "w_sar": np.asarray(W_sar, dtype=np.float16),
        }
        for b in range(B)
    ]
    res = run_bass_kernel_spmd(nc, in_maps, core_ids=list(range(B)))
    out = np.stack(
        [
            res.results[b]["att"].reshape(C, WW, HH).transpose(0, 2, 1)
            for b in range(B)
        ]
    )
    return out.astype(np.float32)
